# revision 8
# baseline (speedup 1.0000x reference)
"""3x3 SAME conv + ReLU on 8 TRN2 cores — hand-semaphored PE pipeline.

Implicit-GEMM mapping: spatial H-shard (28 rows + 1-row halo per core),
channel-major input via xbar transpose DMA, pixel-major PSUM accumulation
over the 9 taps, 450 matmuls of [128x128]x[128,256] bf16 per core inside one
tc.tile_critical() region with hand semaphores and 8 rotating PSUM banks.

Startup path (vs. the previous version): all load DMAs are issued inside the
critical region — the input transpose DMA is split into 8 chunks on the SP
queue with a per-chunk semaphore, and the weight load runs concurrently on
the ACT queue — so the PE stream starts as soon as chunk 0 + weights have
landed (~4 us) instead of after the full serial 1.76 MB + 0.6 MB load
(~11 us). PE warmup runs on a memset tile, so it needs no weight data and
overlaps the loads exactly.
"""

import os
import sys
from contextlib import ExitStack

sys.path.insert(0, "/opt/trn_rl_repo")

FEAT = int(os.environ.get("KFEAT", "3"))

import ml_dtypes
import numpy as np

H = 224
WID = 224
C_IN = 128
C_OUT = 256
KK = 3
NCORES = 8
RPC = H // NCORES
WP = WID + 2
HALO = RPC + 2
NPIX = HALO * WP
T_TILES = 50
YROWS = T_TILES * 128
XROWS = 6864
N_WARM = 10
NBANK = 8
NCHUNK = 8
CHUNK = 864  # xbar src rows must be a multiple of 16; last chunk is 816
CHUNK_LO = [c * CHUNK for c in range(NCHUNK)]
CHUNK_HI = [min((c + 1) * CHUNK, XROWS) for c in range(NCHUNK)]

COMPUTE_DT = ml_dtypes.bfloat16

_COMPILED = None
LAST_RESULT = None


def _need_chunk(t):
    # tile t reads xt columns [t*128 .. t*128 + 2*WP + 2 + 128)
    last_col = t * 128 + 2 * WP + 2 + 128 - 1
    for c in range(NCHUNK):
        if last_col < CHUNK_HI[c]:
            return c
    return NCHUNK - 1


def _build(bench_loop=False):
    import concourse.bacc as bacc
    import concourse.mybir as mybir
    import concourse.tile as tile

    nc = bacc.Bacc("TRN2", target_bir_lowering=False, debug=False, num_devices=NCORES)
    MDT = mybir.dt.from_np(np.dtype(COMPUTE_DT))

    x_d = nc.dram_tensor("xs", [XROWS, C_IN], MDT, kind="ExternalInput").ap()
    w_d = nc.dram_tensor("w", [C_IN, 9 * C_OUT], MDT, kind="ExternalInput").ap()
    if bench_loop:
        r_d = nc.dram_tensor("r", [1, 1], mybir.dt.int32, kind="ExternalInput").ap()
    y_d = nc.dram_tensor("y", [YROWS, C_OUT], MDT, kind="ExternalOutput").ap()

    with ExitStack() as stack:
        banks = [
            stack.enter_context(
                nc.psum_tensor(f"bank{i}", [128, C_OUT], mybir.dt.float32)
            )
            for i in range(NBANK)
        ]
        ots = [
            stack.enter_context(nc.sbuf_tensor(f"ot{i}", [128, C_OUT], MDT))
            for i in range(NBANK)
        ]
        x_sem = stack.enter_context(nc.semaphore("x_sem"))
        w_sem = stack.enter_context(nc.semaphore("w_sem"))
        mm_sem = stack.enter_context(nc.semaphore("mm_sem"))
        act_sem = stack.enter_context(nc.semaphore("act_sem"))
        st_sems = [
            stack.enter_context(nc.semaphore(f"st_sem{i}")) for i in range(NBANK)
        ]

        with tile.TileContext(nc) as tc:
            with (
                tc.tile_pool(name="xt", bufs=1) as xt_pool,
                tc.tile_pool(name="wt", bufs=1) as wt_pool,
            ):
                wm = wt_pool.tile([C_IN, C_OUT], MDT)
                nc.any.memset(wm[:], 0)
                wt = wt_pool.tile([C_IN, 9 * C_OUT], MDT)
                xt = xt_pool.tile([C_IN, XROWS], MDT)
                if bench_loop:
                    rt = wt_pool.tile([1, 1], mybir.dt.int32)
                    nc.sync.dma_start(rt[:], r_d[:])
                    reps = nc.values_load(
                        rt[0:1, 0:1],
                        min_val=0,
                        max_val=1 << 22,
                        skip_runtime_bounds_check=True,
                    )
                def body():
                    # loads issued inside the critical region: x chunks on the
                    # SP queue, weights on the ACT queue in parallel.
                    for c in range(NCHUNK):
                        nc.sync.dma_start(
                            xt[:, CHUNK_LO[c] : CHUNK_HI[c]],
                            x_d[CHUNK_LO[c] : CHUNK_HI[c], :],
                            transpose=True,
                        ).then_inc(x_sem, 16)
                    w_eng = nc.scalar if FEAT >= 3 else nc.sync
                    for wc in range(3):
                        w_eng.dma_start(
                            wt[:, wc * 3 * C_OUT : (wc + 1) * 3 * C_OUT],
                            w_d[:, wc * 3 * C_OUT : (wc + 1) * 3 * C_OUT],
                        ).then_inc(w_sem, 16)
                    # PE clock-gate warmup on the zero tile while loads stream
                    # (writes bank 7, first reused by tile 7 with start=True).
                    for i in range(N_WARM):
                        nc.tensor.matmul(
                            banks[7].ap(), wm[:, 0:128], wm[:, 0:C_OUT],
                            start=(i == 0), stop=(i == N_WARM - 1),
                            skip_group_check=True,
                        )
                    have_chunk = -1
                    for t in range(T_TILES):
                        b = t % NBANK
                        need = _need_chunk(t)
                        if need > have_chunk:
                            if FEAT >= 2:
                                nc.tensor.wait_ge(x_sem, 16 * (need + 1))
                            have_chunk = need
                        if t >= NBANK:
                            nc.tensor.wait_ge(act_sem, t - NBANK + 1)
                        if t == 1:
                            nc.tensor.wait_ge(w_sem, 48)
                        for kh in range(KK):
                            for kw in range(KK):
                                k = kh * KK + kw
                                if t == 0 and k % 3 == 0:
                                    nc.tensor.wait_ge(w_sem, 16 * (k // 3 + 1))
                                off = t * 128 + kh * WP + kw
                                mm = nc.tensor.matmul(
                                    banks[b].ap(),
                                    xt[:, off : off + 128],
                                    wt[:, k * C_OUT : (k + 1) * C_OUT],
                                    start=(k == 0),
                                    stop=(k == 8),
                                    skip_group_check=True,
                                )
                                if k == 8:
                                    mm.then_inc(mm_sem, 1)
                        nc.vector.wait_ge(mm_sem, t + 1)
                        if t >= NBANK:
                            nc.vector.wait_ge(st_sems[b], 16 * (t // NBANK))
                        nc.vector.tensor_scalar_max(
                            ots[b].ap(), banks[b].ap(), 0.0
                        ).then_inc(act_sem, 1)
                        st_eng = nc.scalar if FEAT >= 4 else nc.sync
                        st_eng.wait_ge(act_sem, t + 1)
                        st_eng.dma_start(
                            y_d[t * 128 : (t + 1) * 128, :], ots[b].ap()
                        ).then_inc(st_sems[b], 16)

                if bench_loop:
                    with tc.For_i(0, reps, name="bench"):
                        with tc.tile_critical():
                            body()
                else:
                    with tc.tile_critical():
                        body()

        nc.compile()
    return nc


def _prep_inputs(x: np.ndarray, W: np.ndarray):
    xp = np.zeros((H + 2, WP, C_IN), np.float32)
    xp[1 : H + 1, 1 : WID + 1] = x
    xs = np.zeros((NCORES, XROWS, C_IN), COMPUTE_DT)
    for i in range(NCORES):
        xs[i, 1 : 1 + NPIX] = (
            xp[RPC * i : RPC * i + HALO].reshape(NPIX, C_IN).astype(COMPUTE_DT)
        )
    wh = (
        W.reshape(C_OUT, 9, C_IN)
        .transpose(2, 1, 0)
        .reshape(C_IN, 9 * C_OUT)
        .astype(COMPUTE_DT)
    )
    return xs, wh


def kernel(x: np.ndarray, W: np.ndarray) -> np.ndarray:
    global _COMPILED, LAST_RESULT
    from concourse import bass_utils

    if _COMPILED is None:
        _COMPILED = _build()
    nc = _COMPILED

    xs, wh = _prep_inputs(np.asarray(x, np.float32), np.asarray(W, np.float32))
    in_maps = [{"xs": np.ascontiguousarray(xs[i]), "w": wh} for i in range(NCORES)]

    try:
        res = bass_utils.run_bass_kernel_spmd(nc, in_maps, core_ids=list(range(NCORES)))
    except Exception:
        import os

        if os.environ.get("BASS_TRACE"):
            os.environ.pop("BASS_TRACE", None)
            res = bass_utils.run_bass_kernel_spmd(
                nc, in_maps, core_ids=list(range(NCORES))
            )
        else:
            raise
    LAST_RESULT = res

    y = np.stack([r["y"] for r in res.results])
    y = y[:, : RPC * WP].reshape(NCORES, RPC, WP, C_OUT)[:, :, 1 : WID + 1]
    return y.reshape(H, WID, C_OUT).astype(np.float32)


# revision 9
# speedup vs baseline: 1.0289x; 1.0289x over previous
"""3x3 SAME conv + ReLU on 8 TRN2 cores — hand-semaphored PE pipeline.

Implicit-GEMM mapping: spatial H-shard (28 rows + 1-row halo per core),
channel-major input via xbar transpose DMA, pixel-major PSUM accumulation
over the 9 taps, 450 matmuls of [128x128]x[128,256] bf16 per core inside one
tc.tile_critical() region with hand semaphores and 8 rotating PSUM banks.

Startup path: all load DMAs are issued inside the critical region — the
input transpose DMA is split into 8 chunks on the SP queue with a per-chunk
semaphore, and the weight load is split into 3 tap-group chunks on the ACT
queue with staged waits in tile 0 — so the PE stream starts ~2.7 us in
(after w chunk 1 + x chunk 0 + warmup) instead of after the full serial
1.76 MB + 0.6 MB load (~11 us). PE warmup runs on a memset tile (no weight
dependency) and is sized so the PE never idles between warmup and the real
stream, which would reset the clock-gate ramp.
"""

import os
import sys
from contextlib import ExitStack

sys.path.insert(0, "/opt/trn_rl_repo")

FEAT = int(os.environ.get("KFEAT", "3"))

import ml_dtypes
import numpy as np

H = 224
WID = 224
C_IN = 128
C_OUT = 256
KK = 3
NCORES = 8
RPC = H // NCORES
WP = WID + 2
HALO = RPC + 2
NPIX = HALO * WP
T_TILES = 50
YROWS = T_TILES * 128
XROWS = 6864
N_WARM = 12
NBANK = 8
NCHUNK = 8
CHUNK = 864  # xbar src rows must be a multiple of 16; last chunk is 816
CHUNK_LO = [c * CHUNK for c in range(NCHUNK)]
CHUNK_HI = [min((c + 1) * CHUNK, XROWS) for c in range(NCHUNK)]

COMPUTE_DT = ml_dtypes.bfloat16

_COMPILED = None
LAST_RESULT = None


def _need_chunk(t):
    # tile t reads xt columns [t*128 .. t*128 + 2*WP + 2 + 128)
    last_col = t * 128 + 2 * WP + 2 + 128 - 1
    for c in range(NCHUNK):
        if last_col < CHUNK_HI[c]:
            return c
    return NCHUNK - 1


def _build(bench_loop=False):
    import concourse.bacc as bacc
    import concourse.mybir as mybir
    import concourse.tile as tile

    nc = bacc.Bacc("TRN2", target_bir_lowering=False, debug=False, num_devices=NCORES)
    MDT = mybir.dt.from_np(np.dtype(COMPUTE_DT))

    x_d = nc.dram_tensor("xs", [XROWS, C_IN], MDT, kind="ExternalInput").ap()
    w_d = nc.dram_tensor("w", [C_IN, 9 * C_OUT], MDT, kind="ExternalInput").ap()
    if bench_loop:
        r_d = nc.dram_tensor("r", [1, 1], mybir.dt.int32, kind="ExternalInput").ap()
    y_d = nc.dram_tensor("y", [YROWS, C_OUT], MDT, kind="ExternalOutput").ap()

    with ExitStack() as stack:
        banks = [
            stack.enter_context(
                nc.psum_tensor(f"bank{i}", [128, C_OUT], mybir.dt.float32)
            )
            for i in range(NBANK)
        ]
        ots = [
            stack.enter_context(nc.sbuf_tensor(f"ot{i}", [128, C_OUT], MDT))
            for i in range(NBANK)
        ]
        x_sem = stack.enter_context(nc.semaphore("x_sem"))
        w_sem = stack.enter_context(nc.semaphore("w_sem"))
        mm_sem = stack.enter_context(nc.semaphore("mm_sem"))
        act_sem = stack.enter_context(nc.semaphore("act_sem"))
        st_sems = [
            stack.enter_context(nc.semaphore(f"st_sem{i}")) for i in range(NBANK)
        ]

        with tile.TileContext(nc) as tc:
            with (
                tc.tile_pool(name="xt", bufs=1) as xt_pool,
                tc.tile_pool(name="wt", bufs=1) as wt_pool,
            ):
                wm = wt_pool.tile([C_IN, C_OUT], MDT)
                nc.any.memset(wm[:], 0)
                wt = wt_pool.tile([C_IN, 9 * C_OUT], MDT)
                xt = xt_pool.tile([C_IN, XROWS], MDT)
                if bench_loop:
                    rt = wt_pool.tile([1, 1], mybir.dt.int32)
                    nc.sync.dma_start(rt[:], r_d[:])
                    reps = nc.values_load(
                        rt[0:1, 0:1],
                        min_val=0,
                        max_val=1 << 22,
                        skip_runtime_bounds_check=True,
                    )
                def body():
                    # loads issued inside the critical region: x chunks on the
                    # SP queue, weights on the ACT queue in parallel.
                    for c in range(NCHUNK):
                        nc.sync.dma_start(
                            xt[:, CHUNK_LO[c] : CHUNK_HI[c]],
                            x_d[CHUNK_LO[c] : CHUNK_HI[c], :],
                            transpose=True,
                        ).then_inc(x_sem, 16)
                    w_eng = nc.scalar if FEAT >= 3 else nc.sync
                    for wc in range(3):
                        w_eng.dma_start(
                            wt[:, wc * 3 * C_OUT : (wc + 1) * 3 * C_OUT],
                            w_d[:, wc * 3 * C_OUT : (wc + 1) * 3 * C_OUT],
                        ).then_inc(w_sem, 16)
                    # PE clock-gate warmup on the zero tile while loads stream
                    # (writes bank 7, first reused by tile 7 with start=True).
                    for i in range(N_WARM):
                        nc.tensor.matmul(
                            banks[7].ap(), wm[:, 0:128], wm[:, 0:C_OUT],
                            start=(i == 0), stop=(i == N_WARM - 1),
                            skip_group_check=True,
                        )
                    have_chunk = -1
                    for t in range(T_TILES):
                        b = t % NBANK
                        need = _need_chunk(t)
                        if need > have_chunk:
                            if FEAT >= 2:
                                nc.tensor.wait_ge(x_sem, 16 * (need + 1))
                            have_chunk = need
                        if t >= NBANK:
                            nc.tensor.wait_ge(act_sem, t - NBANK + 1)
                        if t == 1:
                            nc.tensor.wait_ge(w_sem, 48)
                        for kh in range(KK):
                            for kw in range(KK):
                                k = kh * KK + kw
                                if t == 0 and k % 3 == 0:
                                    nc.tensor.wait_ge(w_sem, 16 * (k // 3 + 1))
                                off = t * 128 + kh * WP + kw
                                mm = nc.tensor.matmul(
                                    banks[b].ap(),
                                    xt[:, off : off + 128],
                                    wt[:, k * C_OUT : (k + 1) * C_OUT],
                                    start=(k == 0),
                                    stop=(k == 8),
                                    skip_group_check=True,
                                )
                                if k == 8:
                                    mm.then_inc(mm_sem, 1)
                        nc.vector.wait_ge(mm_sem, t + 1)
                        if t >= NBANK:
                            nc.vector.wait_ge(st_sems[b], 16 * (t // NBANK))
                        nc.vector.tensor_scalar_max(
                            ots[b].ap(), banks[b].ap(), 0.0
                        ).then_inc(act_sem, 1)
                        st_eng = nc.scalar if FEAT >= 4 else nc.sync
                        st_eng.wait_ge(act_sem, t + 1)
                        st_eng.dma_start(
                            y_d[t * 128 : (t + 1) * 128, :], ots[b].ap()
                        ).then_inc(st_sems[b], 16)

                if bench_loop:
                    with tc.For_i(0, reps, name="bench"):
                        with tc.tile_critical():
                            body()
                else:
                    with tc.tile_critical():
                        body()

        nc.compile()
    return nc


def _prep_inputs(x: np.ndarray, W: np.ndarray):
    xp = np.zeros((H + 2, WP, C_IN), np.float32)
    xp[1 : H + 1, 1 : WID + 1] = x
    xs = np.zeros((NCORES, XROWS, C_IN), COMPUTE_DT)
    for i in range(NCORES):
        xs[i, 1 : 1 + NPIX] = (
            xp[RPC * i : RPC * i + HALO].reshape(NPIX, C_IN).astype(COMPUTE_DT)
        )
    wh = (
        W.reshape(C_OUT, 9, C_IN)
        .transpose(2, 1, 0)
        .reshape(C_IN, 9 * C_OUT)
        .astype(COMPUTE_DT)
    )
    return xs, wh


def kernel(x: np.ndarray, W: np.ndarray) -> np.ndarray:
    global _COMPILED, LAST_RESULT
    from concourse import bass_utils

    if _COMPILED is None:
        _COMPILED = _build()
    nc = _COMPILED

    xs, wh = _prep_inputs(np.asarray(x, np.float32), np.asarray(W, np.float32))
    in_maps = [{"xs": np.ascontiguousarray(xs[i]), "w": wh} for i in range(NCORES)]

    try:
        res = bass_utils.run_bass_kernel_spmd(nc, in_maps, core_ids=list(range(NCORES)))
    except Exception:
        import os

        if os.environ.get("BASS_TRACE"):
            os.environ.pop("BASS_TRACE", None)
            res = bass_utils.run_bass_kernel_spmd(
                nc, in_maps, core_ids=list(range(NCORES))
            )
        else:
            raise
    LAST_RESULT = res

    y = np.stack([r["y"] for r in res.results])
    y = y[:, : RPC * WP].reshape(NCORES, RPC, WP, C_OUT)[:, :, 1 : WID + 1]
    return y.reshape(H, WID, C_OUT).astype(np.float32)
